# revision 1
# baseline (speedup 1.0000x reference)
"""Multi-head self-attention Bass/Tile kernel for Trainium2, SPMD over 8 cores.

Problem: B=2, T=4096, D=768, H=12, HD=64 dense MHSA (full TxT scores,
key-padding mask, softmax, out-proj with bias).

Sharding: core c handles batch b=c//4 and query slice q0=(c%4)*1024 for all
12 heads over the full 4096 keys.  No collectives: each core computes a
disjoint [768, 1024] slice of the (transposed) output; the host gathers.

All matmuls contract over the partition dim, so the dataflow is "transposed"
(features on partitions, tokens free):
  phase A: QKV projection.  Q^T per head [64, 1024] stays in SBUF;
           K^T [768, 4096] and V' [12, 4096, 65] staged via DRAM
           (V' carries a ones column per head -> softmax denominator
           falls out of the AV matmul).
  phase B: per head h, per key-tile kt: S[128k, 1024q] = K_h^T.T @ Q_h^T,
           P = exp(S/8 + maskbias_k) on ACT (mask is a per-partition bias),
           O'[65, 512] += V'_kt.T @ P (PSUM accumulation over 32 key tiles).
           Normalize O = O'[0:64] * bcast(1/O'[64]) (PE broadcast matmul).
  phase C: out^T[768, 1024] = Wp^T.T @ O^T + b, DMA out.
"""

import functools
import numpy as np

import concourse.bass as bass
import concourse.mybir as mybir
import concourse.tile as tile
from concourse import bacc
from concourse.bass2jax import (
    _bass_exec_p,
    install_neuronx_cc_hook,
    partition_id_tensor,
)

F32R = mybir.dt.float32r
F32 = mybir.dt.float32
BF16 = mybir.dt.bfloat16
USE_BF16 = True
MMDT = BF16 if USE_BF16 else F32R
AF = mybir.ActivationFunctionType

B, T, D = 2, 4096, 768
H, HD = 12, 64
N_CORES = 8
CORES_PER_B = 4
QS = T // CORES_PER_B          # 1024 query tokens per core
NB = 1e9                        # mask bias magnitude
DT = D // 128                   # 6 d-tiles
KT = T // 128                   # 32 key tiles
QC = QS // 512                  # 2 query chunks of 512


def build_program(reps: int = 1):
    nc = bacc.Bacc("TRN2", target_bir_lowering=False, debug=False,
                   num_devices=N_CORES)

    xT = nc.dram_tensor("xT", [D, T], MMDT, kind="ExternalInput").ap()
    xTq = nc.dram_tensor("xTq", [D, QS], MMDT, kind="ExternalInput").ap()
    wqT = nc.dram_tensor("wqT", [D, D], MMDT, kind="ExternalInput").ap()
    wkT = nc.dram_tensor("wkT", [D, D], MMDT, kind="ExternalInput").ap()
    wvT = nc.dram_tensor("wvT", [D, D], MMDT, kind="ExternalInput").ap()
    wpT = nc.dram_tensor("wpT", [D, D], MMDT, kind="ExternalInput").ap()
    bp = nc.dram_tensor("bp", [128, DT], F32, kind="ExternalInput").ap()
    mb = nc.dram_tensor("mb", [128, KT], F32, kind="ExternalInput").ap()
    onesc = nc.dram_tensor("onesc", [128, H], MMDT, kind="ExternalInput").ap()
    outT = nc.dram_tensor("outT", [D, QS], F32, kind="ExternalOutput").ap()

    KTd = nc.dram_tensor("KTd", [D, T], MMDT).ap()          # K^T staging
    Vp = nc.dram_tensor("Vp", [T, H * (HD + 1)], MMDT).ap()  # V' staging

    with tile.TileContext(nc) as tc, nc.allow_low_precision(
            reason="f32r matmul pipeline"):
        _body(nc, tc, reps, xT, xTq, wqT, wkT, wvT, wpT, bp, mb, onesc,
              outT, KTd, Vp)
    nc.compile()
    return nc


def _body(nc, tc, reps, xT, xTq, wqT, wkT, wvT, wpT, bp, mb, onesc,
          outT, KTd, Vp):
    from contextlib import ExitStack

    with ExitStack() as root:
        const = root.enter_context(tc.tile_pool(name="const", bufs=1))
        mb_sb = const.tile([128, KT], F32, tag="mb")
        nc.sync.dma_start(mb_sb[:], mb[:])
        bp_sb = const.tile([128, DT], F32, tag="bp")
        nc.sync.dma_start(bp_sb[:], bp[:])
        ones64 = const.tile([1, 64], F32, tag="ones64")
        nc.vector.memset(ones64[:], 1.0)
        onesr = const.tile([128, H], MMDT, tag="onesr")
        nc.sync.dma_start(onesr[:], onesc[:])

        # long-lived per-head Q^T and O^T
        qt_pool = root.enter_context(tc.tile_pool(name="qt", bufs=1))
        ot_pool = root.enter_context(tc.tile_pool(name="ot", bufs=1))

        def emit_once():
            qts = _phase_a(nc, tc, qt_pool, xT, xTq, wqT, wkT, wvT, onesr,
                           KTd, Vp)
            ots = _phase_b(nc, tc, ot_pool, qts, mb_sb, ones64, KTd, Vp)
            _phase_c(nc, tc, ots, wpT, bp_sb, outT)

        if reps == 1:
            emit_once()
        elif reps < 0:
            for _ in range(-reps):
                emit_once()
        else:
            with tc.For_i(0, reps, 1):
                emit_once()


def _phase_a(nc, tc, qt_pool, xT, xTq, wqT, wkT, wvT, onesr, KTd, Vp):
    from contextlib import ExitStack

    # --- Q^T projection: per-head tiles [64, QS], SBUF-resident ---
    qts = []
    with ExitStack() as s:
        wq_pool = s.enter_context(tc.tile_pool(name="wq", bufs=1))
        xq_pool = s.enter_context(tc.tile_pool(name="xq", bufs=1))
        qps_pool = s.enter_context(
            tc.tile_pool(name="qps", bufs=2, space="PSUM"))

        wq_sb, xq_sb = [], []
        for d in range(DT):
            w = wq_pool.tile([128, D], MMDT, tag=f"wq{d}")
            nc.sync.dma_start(w[:], wqT[d * 128:(d + 1) * 128, :])
            wq_sb.append(w)
            xq = xq_pool.tile([128, QS], MMDT, tag=f"xq{d}")
            nc.sync.dma_start(xq[:], xTq[d * 128:(d + 1) * 128, :])
            xq_sb.append(xq)

        for h in range(H):
            qt = qt_pool.tile([64, QS], MMDT, tag=f"qt{h}")
            for c in range(QC):
                ps = qps_pool.tile([64, 512], F32, tag="qps")
                for d in range(DT):
                    nc.tensor.matmul(
                        ps[:], wq_sb[d][:, h * 64:(h + 1) * 64],
                        xq_sb[d][:, c * 512:(c + 1) * 512],
                        start=(d == 0), stop=(d == DT - 1))
                nc.vector.tensor_copy(qt[:, c * 512:(c + 1) * 512], ps[:])
            qts.append(qt)

    # --- K^T and V' over the full T, staged to DRAM ---
    with ExitStack() as s:
        wkv_pool = s.enter_context(tc.tile_pool(name="wkv", bufs=1))
        xt_pool = s.enter_context(tc.tile_pool(name="xt", bufs=2))
        stage_pool = s.enter_context(tc.tile_pool(name="stage", bufs=3))
        kps_pool = s.enter_context(
            tc.tile_pool(name="kps", bufs=2, space="PSUM"))
        vps_pool = s.enter_context(
            tc.tile_pool(name="vps", bufs=2, space="PSUM"))

        wk_sb, wv_sb = [], []
        for d in range(DT):
            wk = wkv_pool.tile([128, D], MMDT, tag=f"wk{d}")
            nc.sync.dma_start(wk[:], wkT[d * 128:(d + 1) * 128, :])
            wk_sb.append(wk)
            wv = wkv_pool.tile([128, D], MMDT, tag=f"wv{d}")
            nc.sync.dma_start(wv[:], wvT[d * 128:(d + 1) * 128, :])
            wv_sb.append(wv)

        for tch in range(T // 1024):
            tsl = slice(tch * 1024, (tch + 1) * 1024)
            xt_sb = []
            for d in range(DT):
                xt_t = xt_pool.tile([128, 1024], MMDT, tag=f"xt{d}")
                nc.sync.dma_start(xt_t[:], xT[d * 128:(d + 1) * 128, tsl])
                xt_sb.append(xt_t)

            # K^T rows e*128..e*128+128, cols tsl
            for e in range(DT):
                kst = stage_pool.tile([128, 1024], MMDT, tag="kst")
                for half in range(2):
                    hs = slice(half * 512, (half + 1) * 512)
                    ps = kps_pool.tile([128, 512], F32, tag="kps")
                    for d in range(DT):
                        nc.tensor.matmul(
                            ps[:], wk_sb[d][:, e * 128:(e + 1) * 128],
                            xt_sb[d][:, hs],
                            start=(d == 0), stop=(d == DT - 1))
                    nc.vector.tensor_copy(kst[:, hs], ps[:])
                nc.sync.dma_start(KTd[e * 128:(e + 1) * 128, tsl], kst[:])

            # V natural layout [t, e] + ones col per head
            for tt in range(8):
                t0 = tch * 1024 + tt * 128
                ps = vps_pool.tile([128, D], F32, tag="vps")
                for d in range(DT):
                    lhs = xt_sb[d][:, tt * 128:(tt + 1) * 128]
                    nc.tensor.matmul(ps[:, 0:512], lhs, wv_sb[d][:, 0:512],
                                     start=(d == 0), stop=(d == DT - 1),
                                     skip_group_check=True)
                    nc.tensor.matmul(ps[:, 512:768], lhs, wv_sb[d][:, 512:768],
                                     start=(d == 0), stop=(d == DT - 1),
                                     skip_group_check=True)
                vst = stage_pool.tile([128, H * (HD + 1)], MMDT, tag="vst")
                vst3 = vst[:].rearrange("p (h s) -> p h s", s=HD + 1)
                nc.vector.tensor_copy(
                    vst3[:, :, 0:HD],
                    ps[:].rearrange("p (h s) -> p h s", s=HD))
                nc.vector.tensor_copy(
                    vst3[:, :, HD:HD + 1],
                    onesr[:].rearrange("p (h o) -> p h o", o=1))
                nc.sync.dma_start(Vp[t0:t0 + 128, :], vst[:])
    return qts


def _phase_b(nc, tc, ot_pool, qts, mb_sb, ones64, KTd, Vp):
    from contextlib import ExitStack

    ots = []
    with ExitStack() as s:
        kh_pool = s.enter_context(tc.tile_pool(name="kh", bufs=2))
        vh_pool = s.enter_context(tc.tile_pool(name="vh", bufs=2))
        p_pool = s.enter_context(tc.tile_pool(name="p", bufs=3))
        nrm_pool = s.enter_context(tc.tile_pool(name="nrm", bufs=2))
        sp_pool = s.enter_context(
            tc.tile_pool(name="sp", bufs=2, space="PSUM"))
        op_pool = s.enter_context(
            tc.tile_pool(name="op", bufs=1, space="PSUM"))
        bc_pool = s.enter_context(
            tc.tile_pool(name="bc", bufs=1, space="PSUM"))

        for h in range(H):
            kh = kh_pool.tile([64, T], MMDT, tag="kh")
            nc.sync.dma_start(kh[:], KTd[h * 64:(h + 1) * 64, :])
            vh = vh_pool.tile([128, KT * (HD + 1)], MMDT, tag="vh")
            nc.sync.dma_start(
                vh[:].rearrange("p (kt s) -> p kt s", s=HD + 1),
                Vp.rearrange("(kt p) (h s) -> p kt h s", p=128,
                             s=HD + 1)[:, :, h, :])

            ops = [op_pool.tile([65, 512], F32, tag=f"op{c}", name=f"op{c}")
                   for c in range(QC)]
            for kt in range(KT):
                sp = sp_pool.tile([128, QC * 512], F32, tag="sp")
                for c in range(QC):
                    nc.tensor.matmul(
                        sp[:, c * 512:(c + 1) * 512],
                        kh[:, kt * 128:(kt + 1) * 128],
                        qts[h][:, c * 512:(c + 1) * 512],
                        start=True, stop=True, skip_group_check=True)
                p = p_pool.tile([128, QC * 512], MMDT, tag="p")
                nc.scalar.activation(p[:], sp[:], AF.Exp,
                                     bias=mb_sb[:, kt:kt + 1], scale=0.125)
                for c in range(QC):
                    nc.tensor.matmul(
                        ops[c][:],
                        vh[:, kt * (HD + 1):(kt + 1) * (HD + 1)],
                        p[:, c * 512:(c + 1) * 512],
                        start=(kt == 0), stop=(kt == KT - 1))

            ot = ot_pool.tile([64, QS], MMDT, tag=f"ot{h}")
            for c in range(QC):
                recip = nrm_pool.tile([1, 512], F32, tag="recip")
                nc.vector.reciprocal(recip[:], ops[c][64:65, :])
                bc = bc_pool.tile([64, 512], F32, tag="bc")
                nc.tensor.matmul(bc[:], ones64[:], recip[:],
                                 start=True, stop=True)
                bc_sb = nrm_pool.tile([64, 512], F32, tag="bc_sb")
                nc.vector.tensor_copy(bc_sb[:], bc[:])
                nc.vector.tensor_mul(ot[:, c * 512:(c + 1) * 512],
                                     ops[c][0:64, :], bc_sb[:])
            ots.append(ot)
    return ots


def _phase_c(nc, tc, ots, wpT, bp_sb, outT):
    from contextlib import ExitStack

    with ExitStack() as s:
        wp_pool = s.enter_context(tc.tile_pool(name="wp", bufs=1))
        ost_pool = s.enter_context(tc.tile_pool(name="ost", bufs=3))
        pps_pool = s.enter_context(
            tc.tile_pool(name="pps", bufs=2, space="PSUM"))

        wp_sb = []
        for h in range(H):
            wp = wp_pool.tile([64, D], MMDT, tag=f"wp{h}")
            nc.sync.dma_start(wp[:], wpT[h * 64:(h + 1) * 64, :])
            wp_sb.append(wp)

        for m in range(DT):
            for c in range(QC):
                ps = pps_pool.tile([128, 512], F32, tag="pps")
                for h in range(H):
                    nc.tensor.matmul(
                        ps[:], wp_sb[h][:, m * 128:(m + 1) * 128],
                        ots[h][:, c * 512:(c + 1) * 512],
                        start=(h == 0), stop=(h == H - 1))
                ost = ost_pool.tile([128, 512], F32, tag="ost")
                nc.vector.tensor_scalar_add(ost[:], ps[:], bp_sb[:, m:m + 1])
                nc.sync.dma_start(
                    outT[m * 128:(m + 1) * 128, c * 512:(c + 1) * 512],
                    ost[:])


# ---------------------------------------------------------------- host side

@functools.lru_cache(maxsize=None)
def _get_runner(reps: int = 1):
    import jax
    from jax.sharding import Mesh, PartitionSpec
    from jax.experimental.shard_map import shard_map

    nc = build_program(reps)
    install_neuronx_cc_hook()
    partition_name = (nc.partition_id_tensor.name
                      if nc.partition_id_tensor else None)
    in_names, out_names, out_avals, out_shapes = [], [], [], []
    for alloc in nc.m.functions[0].allocations:
        if not isinstance(alloc, mybir.MemoryLocationSet):
            continue
        name = alloc.memorylocations[0].name
        if alloc.kind == "ExternalInput":
            if name != partition_name:
                in_names.append(name)
        elif alloc.kind == "ExternalOutput":
            out_names.append(name)
            shape = tuple(alloc.tensor_shape)
            dtype = mybir.dt.np(alloc.dtype)
            out_avals.append(jax.core.ShapedArray(shape, dtype))
            out_shapes.append((shape, dtype))
    n_params = len(in_names)
    n_outs = len(out_avals)
    all_in_names = list(in_names) + list(out_names)
    if partition_name is not None:
        all_in_names.append(partition_name)
    donate = tuple(range(n_params, n_params + n_outs))

    def _bodyf(*args):
        operands = list(args)
        if partition_name is not None:
            operands.append(partition_id_tensor())
        outs = _bass_exec_p.bind(
            *operands,
            out_avals=tuple(out_avals),
            in_names=tuple(all_in_names),
            out_names=tuple(out_names),
            lowering_input_output_aliases=(),
            sim_require_finite=True,
            sim_require_nnan=True,
            nc=nc,
        )
        return tuple(outs)

    devices = jax.devices()[:N_CORES]
    mesh = Mesh(np.asarray(devices), ("core",))
    in_specs = (PartitionSpec("core"),) * (n_params + n_outs)
    out_specs = (PartitionSpec("core"),) * len(out_names)
    sharded = jax.jit(
        shard_map(_bodyf, mesh=mesh, in_specs=in_specs, out_specs=out_specs,
                  check_rep=False),
        donate_argnums=donate, keep_unused=True,
    )

    def run(in_maps):
        import jax as _jax
        per_core = [[np.asarray(m[n]) for n in in_names] for m in in_maps]
        concat_in = [np.concatenate([per_core[c][i] for c in range(N_CORES)],
                                    axis=0) for i in range(n_params)]
        concat_zeros = [np.zeros((N_CORES * s[0], *s[1:]), dt)
                        for (s, dt) in out_shapes]
        out_arrs = sharded(*concat_in, *concat_zeros)
        _jax.block_until_ready(out_arrs)
        return [
            {name: np.asarray(out_arrs[i]).reshape(
                N_CORES, *out_shapes[i][0])[c]
             for i, name in enumerate(out_names)}
            for c in range(N_CORES)
        ]

    return run


def make_in_maps(x, mask, w_qkv, w_proj, b_proj):
    import ml_dtypes
    mm_np = ml_dtypes.bfloat16 if USE_BF16 else np.float32
    x = np.asarray(x, np.float32)
    mask = np.asarray(mask)
    w_qkv = np.asarray(w_qkv, np.float32)
    w_proj = np.asarray(w_proj, np.float32)
    b_proj = np.asarray(b_proj, np.float32)

    wqT = np.ascontiguousarray(w_qkv[0:D].T).astype(mm_np)
    wkT = np.ascontiguousarray(w_qkv[D:2 * D].T).astype(mm_np)
    wvT = np.ascontiguousarray(w_qkv[2 * D:3 * D].T).astype(mm_np)
    wpT = np.ascontiguousarray(w_proj.T).astype(mm_np)
    bp = np.ascontiguousarray(b_proj.reshape(DT, 128).T)
    onesc = np.ones((128, H), mm_np)

    xTs = [np.ascontiguousarray(x[b].T).astype(mm_np) for b in range(B)]
    mbs = [np.ascontiguousarray(
        np.where(mask[b], np.float32(-NB), np.float32(0.0))
        .astype(np.float32).reshape(KT, 128).T) for b in range(B)]

    in_maps = []
    for c in range(N_CORES):
        b, qi = divmod(c, CORES_PER_B)
        q0 = qi * QS
        in_maps.append({
            "xT": xTs[b],
            "xTq": np.ascontiguousarray(xTs[b][:, q0:q0 + QS]),
            "wqT": wqT, "wkT": wkT, "wvT": wvT, "wpT": wpT,
            "bp": bp, "mb": mbs[b], "onesc": onesc,
        })
    return in_maps


def assemble_output(results):
    out = np.empty((B, T, D), np.float32)
    for c in range(N_CORES):
        b, qi = divmod(c, CORES_PER_B)
        q0 = qi * QS
        out[b, q0:q0 + QS, :] = results[c]["outT"].T
    return out


def kernel(x, mask, w_qkv, w_proj, b_proj):
    run = _get_runner(1)
    in_maps = make_in_maps(x, mask, w_qkv, w_proj, b_proj)
    results = run(in_maps)
    return assemble_output(results)



# revision 42
# speedup vs baseline: 154.0223x; 154.0223x over previous
"""Multi-head self-attention Bass/Tile kernel for Trainium2, SPMD over 8 cores.

Problem: B=2, T=4096, D=768, H=12, HD=64 dense MHSA (full TxT scores,
key-padding mask, softmax, out-proj with bias).

Sharding: core c handles batch b=c//4 and heads 3*(c%4)..3*(c%4)+2 over ALL
4096 queries (tensor-parallel split of heads).  Each core emits a partial
out-proj sum over its 3 heads' features; the host adds the 4 partials per
batch (equivalent to the all-reduce after proj) and transposes.

Key compaction: the mask is a key-padding mask shared by all queries, so
masked keys (~50%) contribute exactly zero.  The host gathers only unmasked
keys into xcT (padded to a multiple of 128); K/V are projected from the
compacted tokens only, halving S/AV matmul work and exp() work.  Padding
keys get a -1e9 exp-bias so they contribute exactly 0.

Dataflow per core (all SBUF-resident, no DRAM staging):
  A: Q^T per head [64, 4096] (heads pair-packed into [128, 4096] + single),
     K^T compacted [64, Tc] (pair-packed + single),
     V' natural [128t, 65] per (head, key-tile): 64 features + ones column
     (the ones column makes the softmax denominator fall out of AV).
  B: per (query-chunk 512, head): for kt in key tiles:
       S[128k, 512q] = K_tile.T @ Q_chunk  (PSUM)
       P = exp(S/8 + maskbias_k)           (ACT, bf16 SBUF)
       O[128q, 65] += P_qtile.T @ V'_kt    (4 q-tiles, PSUM accumulate)
     normalize per-partition (query): O_n = O[:, :64] * recip(O[:, 64])
     transpose O_n -> O^T [64, 128] tiles (PE), copy into O^T store (Pool).
  C: out^T[128d, 512q] = wpp.T @ O^T_pair + wps.T @ O^T_single (+bias), DMA.

PE saturation: S matmuls run 3 key-tiles ahead of the ACT-dependent AV
matmuls, and independent "filler units" (QKV projection chunks, out-proj
chunks, transposes) are interleaved from a FIFO queue so the tensor engine
never idles (keeps the p-state ramp at full speed).
"""

import functools
import math
import numpy as np

import concourse.bass as bass
import concourse.mybir as mybir
import concourse.tile as tile
from concourse import bacc, masks
from concourse.bass2jax import (
    _bass_exec_p,
    install_neuronx_cc_hook,
    partition_id_tensor,
)

F32 = mybir.dt.float32
BF16 = mybir.dt.bfloat16
FP8 = mybir.dt.float8e4
MMDT = BF16
AF = mybir.ActivationFunctionType

B, T, D = 2, 4096, 768
H, HD = 12, 64
N_CORES = 8
HPC = 3                         # heads per core
DT = D // 128                   # 6 d-tiles
QC = T // 512                   # 8 query chunks of 512
NB = 1e9                        # mask bias magnitude
LOOKAHEAD = 3                   # S-matmul lookahead (key tiles)
EXP_SHIFT = -4.0                # exp(S/8 + EXP_SHIFT): keeps P in fp8 range


def build_program(reps: int = 1, ktc: int = 17):
    """ktc: compacted key tiles (128 keys each).  Padding keys have zeroed
    x columns, so P_pad = exp(0) = 1 and V_pad = 0: only the softmax
    denominator is affected, by exactly n_pad -- corrected via the npad
    input before the reciprocal."""
    nc = bacc.Bacc("TRN2", target_bir_lowering=False, debug=False,
                   num_devices=N_CORES)
    tc_keys = ktc * 128

    xT = nc.dram_tensor("xT", [D, T], MMDT, kind="ExternalInput").ap()
    xcT = nc.dram_tensor("xcT", [D, tc_keys], MMDT, kind="ExternalInput").ap()
    wqp = nc.dram_tensor("wqp", [D, 128], MMDT, kind="ExternalInput").ap()
    wqs = nc.dram_tensor("wqs", [D, 64], MMDT, kind="ExternalInput").ap()
    wkp = nc.dram_tensor("wkp", [D, 128], MMDT, kind="ExternalInput").ap()
    wks = nc.dram_tensor("wks", [D, 64], MMDT, kind="ExternalInput").ap()
    wv = nc.dram_tensor("wv", [D, HPC * HD], MMDT, kind="ExternalInput").ap()
    wpp = nc.dram_tensor("wpp", [128, D], MMDT, kind="ExternalInput").ap()
    wps = nc.dram_tensor("wps", [64, D], MMDT, kind="ExternalInput").ap()
    bp = nc.dram_tensor("bp", [128, DT], F32, kind="ExternalInput").ap()
    npad = nc.dram_tensor("npad", [128, 1], F32, kind="ExternalInput").ap()
    outT = nc.dram_tensor("outT", [D, T], F32, kind="ExternalOutput").ap()

    with tile.TileContext(nc) as tc, nc.allow_low_precision(
            reason="bf16 matmul pipeline"):
        _body(nc, tc, reps, ktc, xT, xcT, wqp, wqs, wkp, wks, wv,
              wpp, wps, bp, npad, outT)
    nc.compile()
    return nc


def _k_chunks(tc_keys):
    out, off = [], 0
    while off < tc_keys:
        cw = min(512, tc_keys - off)
        out.append((off, cw))
        off += cw
    return out


def _body(nc, tc, reps, ktc, xT, xcT, wqp, wqs, wkp, wks, wv,
          wpp, wps, bp, npad, outT):
    from contextlib import ExitStack

    tc_keys = ktc * 128
    WIDE = 3                    # key tiles per wide ACT group

    groups = []                 # lists of key tiles sharing one exp() instr
    for g0 in range(0, ktc, WIDE):
        groups.append(list(range(g0, min(g0 + WIDE, ktc))))
    grp_of = {}
    for gi, kts in enumerate(groups):
        for k in kts:
            grp_of[k] = gi

    with ExitStack() as root:
        # ---- SBUF pools (long-lived) ----
        const = root.enter_context(tc.tile_pool(name="const", bufs=1))
        ident = const.tile([128, 128], MMDT, tag="ident", name="ident")
        masks.make_identity(nc, ident[:])

        wq_sb = const.tile([128, DT * 128], MMDT, tag="wq", name="wq_sb")
        wqs_sb = const.tile([128, DT * 64], MMDT, tag="wqs", name="wqs_sb")
        wk_sb = const.tile([128, DT * 128], MMDT, tag="wk", name="wk_sb")
        wks_sb = const.tile([128, DT * 64], MMDT, tag="wks", name="wks_sb")
        wv_sb = const.tile([128, DT * HPC * HD], MMDT, tag="wv", name="wv_sb")
        wpp_sb = const.tile([128, D], MMDT, tag="wpp", name="wpp_sb")
        wps_sb = const.tile([64, D], MMDT, tag="wps", name="wps_sb")
        bp_sb = const.tile([128, DT], F32, tag="bp", name="bp_sb")
        npad_sb = const.tile([128, 1], F32, tag="npad", name="npad_sb")
        eb_sb = const.tile([128, 1], F32, tag="eb", name="eb_sb")
        ones64 = const.tile([1, 64], F32, tag="ones64", name="ones64")

        xc_pool = root.enter_context(tc.tile_pool(name="xc", bufs=1))
        xt_pool = root.enter_context(tc.tile_pool(name="xt", bufs=1))
        qk_pool = root.enter_context(tc.tile_pool(name="qk", bufs=1))
        v_pool = root.enter_context(tc.tile_pool(name="v", bufs=1))
        ot_pool = root.enter_context(tc.tile_pool(name="ot", bufs=1))
        p_pool = root.enter_context(tc.tile_pool(name="p", bufs=3))
        on_pool = root.enter_context(tc.tile_pool(name="on", bufs=2))
        nrm_pool = root.enter_context(tc.tile_pool(name="nrm", bufs=2))
        ost_pool = root.enter_context(tc.tile_pool(name="ost", bufs=6))
        # ---- PSUM pools: sp(2x3) + op(1) + gen(1) = 8 banks ----
        gen_pool = root.enter_context(
            tc.tile_pool(name="gen", bufs=1, space="PSUM"))
        sp_pool = root.enter_context(
            tc.tile_pool(name="sp", bufs=2, space="PSUM"))
        op_pool = root.enter_context(
            tc.tile_pool(name="op", bufs=1, space="PSUM"))

        def emit_once():
            # long-lived activation tiles; ones-columns set before any DMA
            # so the write order matches emission order for the allocator
            qp_sb = qk_pool.tile([128, T], MMDT, tag="qp", name="qp_sb")
            qs_sb = qk_pool.tile([64, T], MMDT, tag="qs", name="qs_sb")
            kp_sb = qk_pool.tile([128, tc_keys], MMDT, tag="kp",
                                 name="kp_sb")
            ks_sb = qk_pool.tile([64, tc_keys], MMDT, tag="ks", name="ks_sb")
            v_sb = [v_pool.tile([128, ktc * (HD + 1)], MMDT, tag=f"v{h}",
                                name="t_") for h in range(HPC)]
            otp_sb = ot_pool.tile([128, T], MMDT, tag="otp", name="otp_sb")
            ots_sb = ot_pool.tile([64, T], MMDT, tag="ots", name="ots_sb")
            for h in range(HPC):
                v3 = v_sb[h][:].rearrange("p (kt s) -> p kt s", s=HD + 1)
                nc.vector.memset(v3[:, :, HD:HD + 1], 1.0)
            nc.vector.memset(eb_sb[:], EXP_SHIFT)
            nc.vector.memset(ones64[:], 1.0)

            # ------------- input DMAs, need-driven order -------------
            # (HWDGE generation is 625ns per DMA instruction, and transfers
            # serialize on the DMA track -- order by first use.)
            def load_w(dst, dram, w):
                # SBUF APs must be partition-major: iterate (p, d, c)
                nc.sync.dma_start(
                    dst[:].rearrange("p (d c) -> p d c", d=DT),
                    dram.rearrange("(d p) c -> p d c", d=DT))
            kch = _k_chunks(tc_keys)
            xc_sb = [xc_pool.tile([128, tc_keys], MMDT, tag=f"xc{d}",
                                  name="t_") for d in range(DT)]
            xt_sb = {}
            for c in range(QC):
                for d in range(DT):
                    xt_sb[(d, c)] = xt_pool.tile([128, 512], MMDT,
                                                 tag=f"xt{d}q{c}", name="t_")
            load_w(wk_sb, wkp, 128)
            for d in range(DT):
                nc.sync.dma_start(xc_sb[d][:, 0:512],
                                  xcT[d * 128:(d + 1) * 128, 0:512])
            load_w(wv_sb, wv, HPC * HD)
            load_w(wq_sb, wqp, 128)
            for d in range(DT):
                nc.sync.dma_start(xt_sb[(d, 0)][:],
                                  xT[d * 128:(d + 1) * 128, 0:512])
            nc.sync.dma_start(npad_sb[:], npad[:])

            def dma_xt(c):
                qsl = slice(c * 512, (c + 1) * 512)
                def fn():
                    for d in range(DT):
                        nc.sync.dma_start(xt_sb[(d, c)][:],
                                          xT[d * 128:(d + 1) * 128, qsl])
                return fn

            def dma_w_rest():
                load_w(wks_sb, wks, 64)
                load_w(wqs_sb, wqs, 64)
                nc.sync.dma_start(wpp_sb[:], wpp[:])
                nc.sync.dma_start(wps_sb[:], wps[:])
                nc.sync.dma_start(bp_sb[:], bp[:])

            # ------------- long-lived activation tiles -------------

            # ------------- filler-unit queue -------------
            emitted = set()
            queue = []

            def unit(key, fn, front=False):
                if front:
                    queue.insert(0, (key, fn))
                else:
                    queue.append((key, fn))

            def pop_unit():
                if queue:
                    key, fn = queue.pop(0)
                    fn()
                    emitted.add(key)

            def require(*keys):
                while any(k not in emitted for k in keys):
                    assert queue, f"unit queue empty but need {keys}"
                    pop_unit()

            def k_unit(single, ci, off, cw):
                # pair: transposed-layout proj (full-width output).
                # single head: natural-orientation proj (out partitions =
                # tokens, full 128) + PE transpose -- fewer moving rows.
                def fn():
                    if not single:
                        ps = gen_pool.tile([128, 512], F32, tag="gp",
                                           name="ps")
                        for d in range(DT):
                            nc.tensor.matmul(
                                ps[:, 0:cw], wk_sb[:, d * 128:(d + 1) * 128],
                                xc_sb[d][:, off:off + cw],
                                start=(d == 0), stop=(d == DT - 1))
                        nc.vector.tensor_copy(kp_sb[:, off:off + cw],
                                              ps[:, 0:cw])
                        return
                    ps = gen_pool.tile([128, 512], F32, tag="gp", name="ps")
                    ntt = cw // 128
                    for tt in range(ntt):
                        t0 = off + tt * 128
                        for d in range(DT):
                            nc.tensor.matmul(
                                ps[:, tt * 64:tt * 64 + 64],
                                xc_sb[d][:, t0:t0 + 128],
                                wks_sb[:, d * 64:(d + 1) * 64],
                                start=(d == 0), stop=(d == DT - 1),
                                skip_group_check=True)
                    nat = on_pool.tile([128, 256], MMDT, tag="nat",
                                       name="nat")
                    nc.vector.tensor_copy(nat[:, 0:ntt * 64],
                                          ps[:, 0:ntt * 64])
                    g = gen_pool.tile([128, 512], F32, tag="gp", name="g")
                    for tt in range(ntt):
                        tp = g[(tt % 2) * 64:(tt % 2) * 64 + 64,
                               (tt // 2) * 64:(tt // 2) * 64 + 64]
                        tp = tp.bitcast(MMDT)
                        nc.tensor.transpose(
                            tp, nat[:, tt * 64:(tt + 1) * 64], ident[:])
                        nc.gpsimd.tensor_copy(
                            ks_sb[0:64, off + tt * 128:off + (tt + 1) * 128],
                            tp)
                return fn

            def v_unit(h, kt0, nkt):
                def fn():
                    ps = gen_pool.tile([128, 512], F32, tag="gp", name="ps")
                    for j, kt in enumerate(range(kt0, kt0 + nkt)):
                        for d in range(DT):
                            nc.tensor.matmul(
                                ps[:, j * HD:(j + 1) * HD],
                                xc_sb[d][:, kt * 128:(kt + 1) * 128],
                                wv_sb[:, d * HPC * HD + h * HD:
                                      d * HPC * HD + (h + 1) * HD],
                                start=(d == 0), stop=(d == DT - 1),
                                skip_group_check=True)
                    v3 = v_sb[h][:].rearrange("p (kt s) -> p kt s", s=HD + 1)
                    nc.vector.tensor_copy(
                        v3[:, kt0:kt0 + nkt, 0:HD],
                        ps[:, 0:nkt * HD].rearrange("p (kt s) -> p kt s",
                                                    s=HD))
                return fn

            def q_unit(single, c):
                def fn():
                    qsl = slice(c * 512, (c + 1) * 512)  # dst cols
                    if not single:
                        ps = gen_pool.tile([128, 512], F32, tag="gp",
                                           name="ps")
                        for d in range(DT):
                            nc.tensor.matmul(
                                ps[:], wq_sb[:, d * 128:(d + 1) * 128],
                                xt_sb[(d, c)][:],
                                start=(d == 0), stop=(d == DT - 1))
                        nc.vector.tensor_copy(qp_sb[:, qsl], ps[:])
                        return
                    ps = gen_pool.tile([128, 512], F32, tag="gp", name="ps")
                    for tt in range(4):
                        for d in range(DT):
                            nc.tensor.matmul(
                                ps[:, tt * 64:tt * 64 + 64],
                                xt_sb[(d, c)][:, tt * 128:(tt + 1) * 128],
                                wqs_sb[:, d * 64:(d + 1) * 64],
                                start=(d == 0), stop=(d == DT - 1),
                                skip_group_check=True)
                    nat = on_pool.tile([128, 256], MMDT, tag="nat",
                                       name="nat")
                    nc.vector.tensor_copy(nat[:], ps[:, 0:256])
                    g = gen_pool.tile([128, 512], F32, tag="gp", name="g")
                    for tt in range(4):
                        tp = g[(tt % 2) * 64:(tt % 2) * 64 + 64,
                               (tt // 2) * 64:(tt // 2) * 64 + 64]
                        tp = tp.bitcast(MMDT)
                        nc.tensor.transpose(
                            tp, nat[:, tt * 64:(tt + 1) * 64], ident[:])
                        nc.gpsimd.tensor_copy(
                            qs_sb[0:64,
                                  c * 512 + tt * 128:c * 512 + (tt + 1) * 128],
                            tp)
                return fn

            def proj_unit(qc, m, use_sp=False):
                def fn():
                    qsl = slice(qc * 512, (qc + 1) * 512)
                    if use_sp:
                        big = sp_pool.tile([128, WIDE * 512], F32, tag="sp",
                                           name="big")
                        ps = big[:, 0:512]
                    else:
                        ps = gen_pool.tile([128, 512], F32, tag="gp",
                                           name="ps")
                    nc.tensor.matmul(ps[:], wpp_sb[:, m * 128:(m + 1) * 128],
                                     otp_sb[:, qsl], start=True, stop=False)
                    nc.tensor.matmul(ps[:], wps_sb[:, m * 128:(m + 1) * 128],
                                     ots_sb[:, qsl], start=False, stop=True)
                    ost = ost_pool.tile([128, 512], F32, tag="ost",
                                        name="ost")
                    nc.vector.tensor_scalar_add(ost[:], ps[:],
                                                bp_sb[:, m:m + 1])
                    nc.sync.dma_start(outT[m * 128:(m + 1) * 128, qsl],
                                      ost[:])
                return fn

            # initial queue.  The first compute units are force-popped
            # BEFORE the phase-2 DMAs are issued: DMA-completion waits are
            # queue-epoch granular, so any instruction emitted after a DMA
            # block waits for ALL of it.
            vg = [list(range(0, ktc, WIDE)) for _ in range(HPC)]
            unit(("kp", 0), k_unit(False, 0, *kch[0]))
            unit(("v", 0, 0), v_unit(0, 0, WIDE))
            unit(("q", 0, 0), q_unit(False, 0))
            require(("kp", 0), ("v", 0, 0), ("q", 0, 0))
            # phase-2 DMAs: remaining keys + weights (xT stays lazy)
            for d in range(DT):
                nc.sync.dma_start(xc_sb[d][:, 512:tc_keys],
                                  xcT[d * 128:(d + 1) * 128, 512:tc_keys])
            dma_w_rest()
            for i in range(1, len(kch)):
                unit(("kp", i), k_unit(False, i, *kch[i]))
            for kt0 in vg[0][1:]:
                unit(("v", 0, kt0), v_unit(0, kt0, min(WIDE, ktc - kt0)))
            for kt0 in vg[1]:
                unit(("v", 1, kt0), v_unit(1, kt0, min(WIDE, ktc - kt0)))
            for i in range(len(kch)):
                unit(("ks", i), k_unit(True, i, *kch[i]))
            for kt0 in vg[2]:
                unit(("v", 2, kt0), v_unit(2, kt0, min(WIDE, ktc - kt0)))

            def kchunk_of(kt):
                return (kt * 128) // 512

            def vkey(h, kt):
                return ("v", h, (kt // WIDE) * WIDE)

            # ------------- main loop: one flat software pipeline ---------
            # A single global stream over (head, query-chunk, key-tile).
            # S matmuls + exp run LA key-tiles ahead of the AV matmuls,
            # crossing chunk boundaries, so the ACT engine (the pacing
            # resource) never waits for PE to drain a chunk tail.
            LA = 6

            def emit_av(op, h, pt, gi, akt):
                # O'[65, 512q] += V'_kt.T @ P_kt -- a single accumulation
                # group per PSUM bank (interleaved groups within one bank
                # do not accumulate correctly).
                j = akt - groups[gi][0]
                nc.tensor.matmul(
                    op[:],
                    v_sb[h][:, akt * (HD + 1):(akt + 1) * (HD + 1)],
                    pt[:, j * 512:(j + 1) * 512],
                    start=(akt == 0), stop=(akt == ktc - 1),
                    skip_group_check=True)

            def emit_normalize(c, op):
                # denominator row 64 carries +n_pad from the padding keys.
                # The DVE part runs inline; the PE broadcast matmul is
                # deferred via the unit queue so it never stalls the PE
                # stream on the reciprocal.
                den = nrm_pool.tile([1, 512], F32, tag="den", name="den")
                nrm = nrm_pool.tile([1, 512], F32, tag="nrm", name="nrm")
                nc.vector.tensor_scalar_sub(den[:], op[64:65, :],
                                            npad_sb[0:1, 0:1])
                nc.vector.reciprocal(nrm[:], den[:])

                def bc_fn():
                    h, qc = c // QC, c % QC
                    hr, dst = (h * 64, otp_sb) if h < 2 else (0, ots_sb)
                    qsl = slice(qc * 512, (qc + 1) * 512)
                    g = gen_pool.tile([128, 512], F32, tag="gp", name="g")
                    nc.tensor.matmul(g[0:64, :], ones64[:], nrm[:],
                                     start=True, stop=True,
                                     skip_group_check=True)
                    bc = on_pool.tile([64, 512], F32, tag="on", name="bc")
                    nc.vector.tensor_copy(bc[:], g[0:64, :])
                    nc.vector.tensor_mul(dst[hr:hr + 64, qsl], op[0:64, :],
                                         bc[:])
                unit(("bc", c), bc_fn, front=True)

            seq = [(0, qc) for qc in range(QC)]
            seq += [(1, 0), (1, 1)]
            for qc in range(2, QC):
                seq += [(2, qc - 2), (1, qc)]
            seq += [(2, QC - 2), (2, QC - 1)]
            stream = [(h, qc, kt) for h, qc in seq for kt in range(ktc)]
            N = len(stream)
            state = {}
            for n in range(N + LA):
                if n < N:
                    h, qc, kt = stream[n]
                    ci = h * QC + qc
                    if kt == 0:
                        if h == 0 and qc + 1 < QC:
                            dma_xt(qc + 1)()
                    if kt == 8 and h == 0 and qc + 1 < QC:
                        unit(("q", 0, qc + 1), q_unit(False, qc + 1),
                             front=True)
                    if kt == 12 and h == 0:
                        unit(("q", 1, qc), q_unit(True, qc), front=True)
                    if kt == 0:
                        require(("q", 0, qc) if h < 2 else ("q", 1, qc))
                        state[ci] = {
                            "op": None, "sp": {}, "p": {}, "h": h, "qc": qc,
                        }
                    st = state[ci]
                    kkey = "kp" if h < 2 else "ks"
                    ksb = kp_sb if h < 2 else ks_sb
                    qsb = qp_sb if h < 2 else qs_sb
                    hr = h * 64 if h < 2 else 0
                    qsl = slice(qc * 512, (qc + 1) * 512)
                    gi = grp_of[kt]
                    kts = groups[gi]
                    if kt == kts[0]:
                        st["sp"][gi] = sp_pool.tile([128, WIDE * 512], F32,
                                                    tag="sp", name="sp")
                        st["p"][gi] = p_pool.tile([128, WIDE * 512], MMDT,
                                                  tag="p", name="pt")
                    require((kkey, kchunk_of(kt)))
                    sp = st["sp"][gi]
                    scol = (kt - kts[0]) * 512
                    nc.tensor.matmul(
                        sp[:, scol:scol + 512],
                        ksb[hr:hr + 64, kt * 128:(kt + 1) * 128],
                        qsb[hr:hr + 64, qsl],
                        start=True, stop=True, skip_group_check=True)
                    if kt == kts[-1]:
                        nc.scalar.activation(
                            st["p"][gi][:, 0:len(kts) * 512],
                            sp[:, 0:len(kts) * 512], AF.Exp,
                            bias=eb_sb[:, 0:1], scale=0.125)
                m = n - LA
                if m >= 0:
                    h2_, qc2_, akt = stream[m]
                    cj = h2_ * QC + qc2_
                    st = state[cj]
                    require(vkey(h2_, akt))
                    gj = grp_of[akt]
                    if st["op"] is None:
                        # allocated at first use: with bufs=1 the previous
                        # chunk's tail AVs must all be emitted before this
                        # alias of the same PSUM bank exists
                        st["op"] = op_pool.tile([65, 512], F32, tag="op",
                                                name="op")
                    emit_av(st["op"], h2_, st["p"][gj], gj, akt)
                    if akt == ktc - 1:
                        emit_normalize(cj, st["op"])
                        if h2_ == 2:
                            for mm in range(DT):
                                unit(("proj", qc2_, mm),
                                     proj_unit(qc2_, mm,
                                               use_sp=qc2_ >= QC - 1))
                        del state[cj]
                if (n % ktc) in (3, 6, 9, 12, 15):
                    pop_unit()
            # drain
            while queue:
                pop_unit()

        if reps == 1:
            emit_once()
        elif reps < 0:
            for _ in range(-reps):
                emit_once()
        else:
            with tc.For_i(0, reps, 1):
                emit_once()


# ---------------------------------------------------------------- host side

@functools.lru_cache(maxsize=None)
def _get_runner(reps: int, ktc: int):
    import jax
    from jax.sharding import Mesh, PartitionSpec
    from jax.experimental.shard_map import shard_map

    nc = build_program(reps, ktc)
    install_neuronx_cc_hook()
    partition_name = (nc.partition_id_tensor.name
                      if nc.partition_id_tensor else None)
    in_names, out_names, out_avals, out_shapes = [], [], [], []
    for alloc in nc.m.functions[0].allocations:
        if not isinstance(alloc, mybir.MemoryLocationSet):
            continue
        name = alloc.memorylocations[0].name
        if alloc.kind == "ExternalInput":
            if name != partition_name:
                in_names.append(name)
        elif alloc.kind == "ExternalOutput":
            out_names.append(name)
            shape = tuple(alloc.tensor_shape)
            dtype = mybir.dt.np(alloc.dtype)
            out_avals.append(jax.core.ShapedArray(shape, dtype))
            out_shapes.append((shape, dtype))
    n_params = len(in_names)
    n_outs = len(out_avals)
    all_in_names = list(in_names) + list(out_names)
    if partition_name is not None:
        all_in_names.append(partition_name)
    donate = tuple(range(n_params, n_params + n_outs))

    def _bodyf(*args):
        operands = list(args)
        if partition_name is not None:
            operands.append(partition_id_tensor())
        outs = _bass_exec_p.bind(
            *operands,
            out_avals=tuple(out_avals),
            in_names=tuple(all_in_names),
            out_names=tuple(out_names),
            lowering_input_output_aliases=(),
            sim_require_finite=True,
            sim_require_nnan=True,
            nc=nc,
        )
        return tuple(outs)

    devices = jax.devices()[:N_CORES]
    mesh = Mesh(np.asarray(devices), ("core",))
    in_specs = (PartitionSpec("core"),) * (n_params + n_outs)
    out_specs = (PartitionSpec("core"),) * len(out_names)
    sharded = jax.jit(
        shard_map(_bodyf, mesh=mesh, in_specs=in_specs, out_specs=out_specs,
                  check_rep=False),
        donate_argnums=donate, keep_unused=True,
    )

    def run(in_maps):
        import jax as _jax
        per_core = [[np.asarray(m[n]) for n in in_names] for m in in_maps]
        concat_in = [np.concatenate([per_core[c][i] for c in range(N_CORES)],
                                    axis=0) for i in range(n_params)]
        concat_zeros = [np.zeros((N_CORES * s[0], *s[1:]), dt)
                        for (s, dt) in out_shapes]
        out_arrs = sharded(*concat_in, *concat_zeros)
        _jax.block_until_ready(out_arrs)
        return [
            {name: np.asarray(out_arrs[i]).reshape(
                N_CORES, *out_shapes[i][0])[c]
             for i, name in enumerate(out_names)}
            for c in range(N_CORES)
        ]

    return run


def _plan_compaction(mask):
    mask = np.asarray(mask)
    keeps = [np.where(~mask[b])[0] for b in range(B)]
    ktc = max(1, -(-max(len(k) for k in keeps) // 128))
    return keeps, ktc


def make_in_maps(x, mask, w_qkv, w_proj, b_proj, keeps=None, ktc=None):
    import ml_dtypes
    mm_np = ml_dtypes.bfloat16
    x = np.asarray(x, np.float32)
    mask = np.asarray(mask)
    w_qkv = np.asarray(w_qkv, np.float32)
    w_proj = np.asarray(w_proj, np.float32)
    b_proj = np.asarray(b_proj, np.float32)
    if keeps is None:
        keeps, ktc = _plan_compaction(mask)
    tc_keys = ktc * 128

    xTs, xcTs, mbs = [], [], []
    for b in range(B):
        xTs.append(np.ascontiguousarray(x[b].T).astype(mm_np))
        xc = np.zeros((tc_keys, D), np.float32)
        xc[:len(keeps[b])] = x[b][keeps[b]]
        xcTs.append(np.ascontiguousarray(xc.T).astype(mm_np))
        # padding keys contribute exp(0/8 + EXP_SHIFT) each, as rounded to
        # fp8e4m3 by the activation output
        p_pad = float(np.float32(
            ml_dtypes.float8_e4m3(np.exp(np.float32(EXP_SHIFT)))))
        mbs.append(np.full((128, 1), (tc_keys - len(keeps[b])) * p_pad,
                           np.float32))

    bp_real = np.ascontiguousarray(b_proj.reshape(DT, 128).T)
    bp_zero = np.zeros_like(bp_real)

    in_maps = []
    for c in range(N_CORES):
        b, hg = divmod(c, 4)
        hs = [hg * HPC + i for i in range(HPC)]
        qrows = np.r_[hs[0] * 64:(hs[0] + 1) * 64, hs[1] * 64:(hs[1] + 1) * 64]
        srow = slice(hs[2] * 64, (hs[2] + 1) * 64)
        feat = np.r_[tuple(np.r_[h * 64:(h + 1) * 64] for h in hs)]
        in_maps.append({
            "xT": xTs[b],
            "xcT": xcTs[b],
            "wqp": np.ascontiguousarray(w_qkv[qrows, :].T).astype(mm_np),
            "wqs": np.ascontiguousarray(w_qkv[srow, :].T).astype(mm_np),
            "wkp": np.ascontiguousarray(w_qkv[D + qrows, :].T).astype(mm_np),
            "wks": np.ascontiguousarray(
                w_qkv[D + hs[2] * 64:D + (hs[2] + 1) * 64, :].T).astype(mm_np),
            "wv": np.ascontiguousarray(
                w_qkv[2 * D + feat, :].T).astype(mm_np),
            "wpp": np.ascontiguousarray(w_proj[:, feat[:128]].T).astype(mm_np),
            "wps": np.ascontiguousarray(w_proj[:, feat[128:]].T).astype(mm_np),
            "bp": bp_real if hg == 0 else bp_zero,
            "npad": mbs[b],
        })
    return in_maps


def assemble_output(results):
    out = np.empty((B, T, D), np.float32)
    for b in range(B):
        acc = results[4 * b]["outT"].astype(np.float32)
        for hg in range(1, 4):
            acc = acc + results[4 * b + hg]["outT"]
        out[b] = acc.T
    return out


def kernel(x, mask, w_qkv, w_proj, b_proj):
    keeps, ktc = _plan_compaction(mask)
    run = _get_runner(1, ktc)
    in_maps = make_in_maps(x, mask, w_qkv, w_proj, b_proj, keeps, ktc)
    results = run(in_maps)
    return assemble_output(results)


# revision 47
# speedup vs baseline: 158.5963x; 1.0297x over previous
"""Multi-head self-attention Bass/Tile kernel for Trainium2, SPMD over 8 cores.

Problem: B=2, T=4096, D=768, H=12, HD=64 dense MHSA (full TxT scores,
key-padding mask, softmax, out-proj with bias).

Sharding: core c handles batch b=c//4 and heads 3*(c%4)..3*(c%4)+2 over ALL
4096 queries (tensor-parallel split of heads).  Each core emits a partial
out-proj sum over its 3 heads' features; the host adds the 4 partials per
batch (equivalent to the all-reduce after proj) and transposes.

Key compaction: the mask is a key-padding mask shared by all queries, so
masked keys (~50%) contribute exactly zero.  The host gathers only unmasked
keys into xcT (padded to a multiple of 128); K/V are projected from the
compacted tokens only, halving S/AV matmul work and exp() work.  Padding
keys get a -1e9 exp-bias so they contribute exactly 0.

Dataflow per core (all SBUF-resident, no DRAM staging):
  A: Q^T per head [64, 4096] (heads pair-packed into [128, 4096] + single),
     K^T compacted [64, Tc] (pair-packed + single),
     V' natural [128t, 65] per (head, key-tile): 64 features + ones column
     (the ones column makes the softmax denominator fall out of AV).
  B: per (query-chunk 512, head): for kt in key tiles:
       S[128k, 512q] = K_tile.T @ Q_chunk  (PSUM)
       P = exp(S/8 + maskbias_k)           (ACT, bf16 SBUF)
       O[128q, 65] += P_qtile.T @ V'_kt    (4 q-tiles, PSUM accumulate)
     normalize per-partition (query): O_n = O[:, :64] * recip(O[:, 64])
     transpose O_n -> O^T [64, 128] tiles (PE), copy into O^T store (Pool).
  C: out^T[128d, 512q] = wpp.T @ O^T_pair + wps.T @ O^T_single (+bias), DMA.

PE saturation: S matmuls run 3 key-tiles ahead of the ACT-dependent AV
matmuls, and independent "filler units" (QKV projection chunks, out-proj
chunks, transposes) are interleaved from a FIFO queue so the tensor engine
never idles (keeps the p-state ramp at full speed).
"""

import functools
import math
import numpy as np

import concourse.bass as bass
import concourse.mybir as mybir
import concourse.tile as tile
from concourse import bacc, masks
from concourse.bass2jax import (
    _bass_exec_p,
    install_neuronx_cc_hook,
    partition_id_tensor,
)

F32 = mybir.dt.float32
BF16 = mybir.dt.bfloat16
FP8 = mybir.dt.float8e4
MMDT = BF16
AF = mybir.ActivationFunctionType

B, T, D = 2, 4096, 768
H, HD = 12, 64
N_CORES = 8
HPC = 3                         # heads per core
DT = D // 128                   # 6 d-tiles
QC = T // 512                   # 8 query chunks of 512
NB = 1e9                        # mask bias magnitude
LOOKAHEAD = 3                   # S-matmul lookahead (key tiles)
EXP_SHIFT = -4.0                # exp(S/8 + EXP_SHIFT): keeps P in fp8 range


def build_program(reps: int = 1, ktc: int = 17):
    """ktc: compacted key tiles (128 keys each).  Padding keys have zeroed
    x columns, so P_pad = exp(0) = 1 and V_pad = 0: only the softmax
    denominator is affected, by exactly n_pad -- corrected via the npad
    input before the reciprocal."""
    nc = bacc.Bacc("TRN2", target_bir_lowering=False, debug=False,
                   num_devices=N_CORES)
    tc_keys = ktc * 128

    xT = nc.dram_tensor("xT", [D, T], MMDT, kind="ExternalInput").ap()
    xcT = nc.dram_tensor("xcT", [D, tc_keys], MMDT, kind="ExternalInput").ap()
    wqp = nc.dram_tensor("wqp", [D, 128], MMDT, kind="ExternalInput").ap()
    wqs = nc.dram_tensor("wqs", [D, 64], MMDT, kind="ExternalInput").ap()
    wkp = nc.dram_tensor("wkp", [D, 128], MMDT, kind="ExternalInput").ap()
    wks = nc.dram_tensor("wks", [D, 64], MMDT, kind="ExternalInput").ap()
    wv = nc.dram_tensor("wv", [D, HPC * HD], MMDT, kind="ExternalInput").ap()
    wpp = nc.dram_tensor("wpp", [128, D], MMDT, kind="ExternalInput").ap()
    wps = nc.dram_tensor("wps", [64, D], MMDT, kind="ExternalInput").ap()
    bp = nc.dram_tensor("bp", [128, DT], F32, kind="ExternalInput").ap()
    npad = nc.dram_tensor("npad", [128, 1], F32, kind="ExternalInput").ap()
    outT = nc.dram_tensor("outT", [D, T], F32, kind="ExternalOutput").ap()

    with tile.TileContext(nc) as tc, nc.allow_low_precision(
            reason="bf16 matmul pipeline"):
        _body(nc, tc, reps, ktc, xT, xcT, wqp, wqs, wkp, wks, wv,
              wpp, wps, bp, npad, outT)
    nc.compile()
    return nc


def _k_chunks(tc_keys):
    out, off = [], 0
    while off < tc_keys:
        cw = min(512, tc_keys - off)
        out.append((off, cw))
        off += cw
    return out


def _body(nc, tc, reps, ktc, xT, xcT, wqp, wqs, wkp, wks, wv,
          wpp, wps, bp, npad, outT):
    from contextlib import ExitStack

    tc_keys = ktc * 128
    WIDE = 3                    # key tiles per wide ACT group

    groups = []                 # lists of key tiles sharing one exp() instr
    for g0 in range(0, ktc, WIDE):
        groups.append(list(range(g0, min(g0 + WIDE, ktc))))
    grp_of = {}
    for gi, kts in enumerate(groups):
        for k in kts:
            grp_of[k] = gi

    with ExitStack() as root:
        # ---- SBUF pools (long-lived) ----
        const = root.enter_context(tc.tile_pool(name="const", bufs=1))
        ident = const.tile([128, 128], MMDT, tag="ident", name="ident")
        masks.make_identity(nc, ident[:])

        wq_sb = const.tile([128, DT * 128], MMDT, tag="wq", name="wq_sb")
        wqs_sb = const.tile([128, DT * 64], MMDT, tag="wqs", name="wqs_sb")
        wk_sb = const.tile([128, DT * 128], MMDT, tag="wk", name="wk_sb")
        wks_sb = const.tile([128, DT * 64], MMDT, tag="wks", name="wks_sb")
        wv_sb = const.tile([128, DT * HPC * HD], MMDT, tag="wv", name="wv_sb")
        wpp_sb = const.tile([128, D], MMDT, tag="wpp", name="wpp_sb")
        wps_sb = const.tile([64, D], MMDT, tag="wps", name="wps_sb")
        bp_sb = const.tile([128, DT], F32, tag="bp", name="bp_sb")
        npad_sb = const.tile([128, 1], F32, tag="npad", name="npad_sb")
        eb_sb = const.tile([128, 1], F32, tag="eb", name="eb_sb")
        ones64 = const.tile([1, 64], F32, tag="ones64", name="ones64")

        xc_pool = root.enter_context(tc.tile_pool(name="xc", bufs=1))
        xt_pool = root.enter_context(tc.tile_pool(name="xt", bufs=1))
        qk_pool = root.enter_context(tc.tile_pool(name="qk", bufs=1))
        v_pool = root.enter_context(tc.tile_pool(name="v", bufs=1))
        ot_pool = root.enter_context(tc.tile_pool(name="ot", bufs=1))
        p_pool = root.enter_context(tc.tile_pool(name="p", bufs=4))
        on_pool = root.enter_context(tc.tile_pool(name="on", bufs=2))
        nrm_pool = root.enter_context(tc.tile_pool(name="nrm", bufs=2))
        ost_pool = root.enter_context(tc.tile_pool(name="ost", bufs=6))
        # ---- PSUM pools: sp(2x3) + op(1) + gen(1) = 8 banks ----
        gen_pool = root.enter_context(
            tc.tile_pool(name="gen", bufs=1, space="PSUM"))
        sp_pool = root.enter_context(
            tc.tile_pool(name="sp", bufs=2, space="PSUM"))
        op_pool = root.enter_context(
            tc.tile_pool(name="op", bufs=1, space="PSUM"))

        def emit_once():
            # long-lived activation tiles; ones-columns set before any DMA
            # so the write order matches emission order for the allocator
            qp_sb = qk_pool.tile([128, T], MMDT, tag="qp", name="qp_sb")
            qs_sb = qk_pool.tile([64, T], MMDT, tag="qs", name="qs_sb")
            kp_sb = qk_pool.tile([128, tc_keys], MMDT, tag="kp",
                                 name="kp_sb")
            ks_sb = qk_pool.tile([64, tc_keys], MMDT, tag="ks", name="ks_sb")
            v_sb = [v_pool.tile([128, ktc * (HD + 1)], MMDT, tag=f"v{h}",
                                name="t_") for h in range(HPC)]
            otp_sb = ot_pool.tile([128, T], MMDT, tag="otp", name="otp_sb")
            ots_sb = ot_pool.tile([64, T], MMDT, tag="ots", name="ots_sb")
            for h in range(HPC):
                v3 = v_sb[h][:].rearrange("p (kt s) -> p kt s", s=HD + 1)
                nc.vector.memset(v3[:, :, HD:HD + 1], 1.0)
            nc.vector.memset(eb_sb[:], EXP_SHIFT)
            nc.vector.memset(ones64[:], 1.0)

            # ------------- input DMAs, need-driven order -------------
            # (HWDGE generation is 625ns per DMA instruction, and transfers
            # serialize on the DMA track -- order by first use.)
            def load_w(dst, dram, w):
                # SBUF APs must be partition-major: iterate (p, d, c)
                nc.sync.dma_start(
                    dst[:].rearrange("p (d c) -> p d c", d=DT),
                    dram.rearrange("(d p) c -> p d c", d=DT))
            kch = _k_chunks(tc_keys)
            xc_sb = [xc_pool.tile([128, tc_keys], MMDT, tag=f"xc{d}",
                                  name="t_") for d in range(DT)]
            xt_sb = {}
            for c in range(QC):
                for d in range(DT):
                    xt_sb[(d, c)] = xt_pool.tile([128, 512], MMDT,
                                                 tag=f"xt{d}q{c}", name="t_")
            load_w(wk_sb, wkp, 128)
            for d in range(DT):
                nc.sync.dma_start(xc_sb[d][:, 0:512],
                                  xcT[d * 128:(d + 1) * 128, 0:512])
            load_w(wv_sb, wv, HPC * HD)
            load_w(wq_sb, wqp, 128)
            for d in range(DT):
                nc.sync.dma_start(xt_sb[(d, 0)][:],
                                  xT[d * 128:(d + 1) * 128, 0:512])
            nc.sync.dma_start(npad_sb[:], npad[:])

            def dma_xt(c):
                qsl = slice(c * 512, (c + 1) * 512)
                def fn():
                    for d in range(DT):
                        nc.sync.dma_start(xt_sb[(d, c)][:],
                                          xT[d * 128:(d + 1) * 128, qsl])
                return fn

            def dma_w_rest():
                load_w(wks_sb, wks, 64)
                load_w(wqs_sb, wqs, 64)
                nc.sync.dma_start(wpp_sb[:], wpp[:])
                nc.sync.dma_start(wps_sb[:], wps[:])
                nc.sync.dma_start(bp_sb[:], bp[:])

            # ------------- long-lived activation tiles -------------

            # ------------- filler-unit queue -------------
            emitted = set()
            queue = []

            def unit(key, fn, front=False):
                if front:
                    queue.insert(0, (key, fn))
                else:
                    queue.append((key, fn))

            def pop_unit():
                if queue:
                    key, fn = queue.pop(0)
                    fn()
                    emitted.add(key)

            def require(*keys):
                while any(k not in emitted for k in keys):
                    assert queue, f"unit queue empty but need {keys}"
                    pop_unit()

            def k_unit(single, ci, off, cw):
                # pair: transposed-layout proj (full-width output).
                # single head: natural-orientation proj (out partitions =
                # tokens, full 128) + PE transpose -- fewer moving rows.
                def fn():
                    if not single:
                        ps = gen_pool.tile([128, 512], F32, tag="gp",
                                           name="ps")
                        for d in range(DT):
                            nc.tensor.matmul(
                                ps[:, 0:cw], wk_sb[:, d * 128:(d + 1) * 128],
                                xc_sb[d][:, off:off + cw],
                                start=(d == 0), stop=(d == DT - 1))
                        nc.vector.tensor_copy(kp_sb[:, off:off + cw],
                                              ps[:, 0:cw])
                        return
                    ps = gen_pool.tile([128, 512], F32, tag="gp", name="ps")
                    ntt = cw // 128
                    for tt in range(ntt):
                        t0 = off + tt * 128
                        for d in range(DT):
                            nc.tensor.matmul(
                                ps[:, tt * 64:tt * 64 + 64],
                                xc_sb[d][:, t0:t0 + 128],
                                wks_sb[:, d * 64:(d + 1) * 64],
                                start=(d == 0), stop=(d == DT - 1),
                                skip_group_check=True)
                    nat = on_pool.tile([128, 256], MMDT, tag="nat",
                                       name="nat")
                    nc.vector.tensor_copy(nat[:, 0:ntt * 64],
                                          ps[:, 0:ntt * 64])
                    g = gen_pool.tile([128, 512], F32, tag="gp", name="g")
                    for tt in range(ntt):
                        tp = g[(tt % 2) * 64:(tt % 2) * 64 + 64,
                               (tt // 2) * 64:(tt // 2) * 64 + 64]
                        tp = tp.bitcast(MMDT)
                        nc.tensor.transpose(
                            tp, nat[:, tt * 64:(tt + 1) * 64], ident[:])
                        nc.gpsimd.tensor_copy(
                            ks_sb[0:64, off + tt * 128:off + (tt + 1) * 128],
                            tp)
                return fn

            def v_unit(h, kt0, nkt):
                def fn():
                    ps = gen_pool.tile([128, 512], F32, tag="gp", name="ps")
                    for j, kt in enumerate(range(kt0, kt0 + nkt)):
                        for d in range(DT):
                            nc.tensor.matmul(
                                ps[:, j * HD:(j + 1) * HD],
                                xc_sb[d][:, kt * 128:(kt + 1) * 128],
                                wv_sb[:, d * HPC * HD + h * HD:
                                      d * HPC * HD + (h + 1) * HD],
                                start=(d == 0), stop=(d == DT - 1),
                                skip_group_check=True)
                    v3 = v_sb[h][:].rearrange("p (kt s) -> p kt s", s=HD + 1)
                    nc.vector.tensor_copy(
                        v3[:, kt0:kt0 + nkt, 0:HD],
                        ps[:, 0:nkt * HD].rearrange("p (kt s) -> p kt s",
                                                    s=HD))
                return fn

            def q_unit(single, c):
                def fn():
                    qsl = slice(c * 512, (c + 1) * 512)  # dst cols
                    if not single:
                        ps = gen_pool.tile([128, 512], F32, tag="gp",
                                           name="ps")
                        for d in range(DT):
                            nc.tensor.matmul(
                                ps[:], wq_sb[:, d * 128:(d + 1) * 128],
                                xt_sb[(d, c)][:],
                                start=(d == 0), stop=(d == DT - 1))
                        nc.vector.tensor_copy(qp_sb[:, qsl], ps[:])
                        return
                    ps = gen_pool.tile([128, 512], F32, tag="gp", name="ps")
                    for tt in range(4):
                        for d in range(DT):
                            nc.tensor.matmul(
                                ps[:, tt * 64:tt * 64 + 64],
                                xt_sb[(d, c)][:, tt * 128:(tt + 1) * 128],
                                wqs_sb[:, d * 64:(d + 1) * 64],
                                start=(d == 0), stop=(d == DT - 1),
                                skip_group_check=True)
                    nat = on_pool.tile([128, 256], MMDT, tag="nat",
                                       name="nat")
                    nc.vector.tensor_copy(nat[:], ps[:, 0:256])
                    g = gen_pool.tile([128, 512], F32, tag="gp", name="g")
                    for tt in range(4):
                        tp = g[(tt % 2) * 64:(tt % 2) * 64 + 64,
                               (tt // 2) * 64:(tt // 2) * 64 + 64]
                        tp = tp.bitcast(MMDT)
                        nc.tensor.transpose(
                            tp, nat[:, tt * 64:(tt + 1) * 64], ident[:])
                        nc.gpsimd.tensor_copy(
                            qs_sb[0:64,
                                  c * 512 + tt * 128:c * 512 + (tt + 1) * 128],
                            tp)
                return fn

            def proj_unit(qc, m, use_sp=False):
                def fn():
                    qsl = slice(qc * 512, (qc + 1) * 512)
                    if use_sp:
                        big = sp_pool.tile([128, WIDE * 512], F32, tag="sp",
                                           name="big")
                        ps = big[:, 0:512]
                    else:
                        ps = gen_pool.tile([128, 512], F32, tag="gp",
                                           name="ps")
                    nc.tensor.matmul(ps[:], wpp_sb[:, m * 128:(m + 1) * 128],
                                     otp_sb[:, qsl], start=True, stop=False)
                    nc.tensor.matmul(ps[:], wps_sb[:, m * 128:(m + 1) * 128],
                                     ots_sb[:, qsl], start=False, stop=True)
                    ost = ost_pool.tile([128, 512], F32, tag="ost",
                                        name="ost")
                    nc.vector.tensor_scalar_add(ost[:], ps[:],
                                                bp_sb[:, m:m + 1])
                    nc.sync.dma_start(outT[m * 128:(m + 1) * 128, qsl],
                                      ost[:])
                return fn

            # initial queue.  The first compute units are force-popped
            # BEFORE the phase-2 DMAs are issued: DMA-completion waits are
            # queue-epoch granular, so any instruction emitted after a DMA
            # block waits for ALL of it.
            vg = [list(range(0, ktc, WIDE)) for _ in range(HPC)]
            unit(("kp", 0), k_unit(False, 0, *kch[0]))
            unit(("v", 0, 0), v_unit(0, 0, WIDE))
            unit(("q", 0, 0), q_unit(False, 0))
            require(("kp", 0), ("v", 0, 0), ("q", 0, 0))
            # phase-2 DMAs: remaining keys + weights (xT stays lazy)
            for d in range(DT):
                nc.sync.dma_start(xc_sb[d][:, 512:tc_keys],
                                  xcT[d * 128:(d + 1) * 128, 512:tc_keys])
            dma_w_rest()
            for i in range(1, len(kch)):
                unit(("kp", i), k_unit(False, i, *kch[i]))
            for kt0 in vg[0][1:]:
                unit(("v", 0, kt0), v_unit(0, kt0, min(WIDE, ktc - kt0)))
            for kt0 in vg[1]:
                unit(("v", 1, kt0), v_unit(1, kt0, min(WIDE, ktc - kt0)))
            for i in range(len(kch)):
                unit(("ks", i), k_unit(True, i, *kch[i]))
            for kt0 in vg[2]:
                unit(("v", 2, kt0), v_unit(2, kt0, min(WIDE, ktc - kt0)))

            def kchunk_of(kt):
                return (kt * 128) // 512

            def vkey(h, kt):
                return ("v", h, (kt // WIDE) * WIDE)

            # ------------- main loop: one flat software pipeline ---------
            # A single global stream over (head, query-chunk, key-tile).
            # S matmuls + exp run LA key-tiles ahead of the AV matmuls,
            # crossing chunk boundaries, so the ACT engine (the pacing
            # resource) never waits for PE to drain a chunk tail.
            LA = 7

            def emit_av(op, h, pt, gi, akt):
                # O'[65, 512q] += V'_kt.T @ P_kt -- a single accumulation
                # group per PSUM bank (interleaved groups within one bank
                # do not accumulate correctly).
                j = akt - groups[gi][0]
                nc.tensor.matmul(
                    op[:],
                    v_sb[h][:, akt * (HD + 1):(akt + 1) * (HD + 1)],
                    pt[:, j * 512:(j + 1) * 512],
                    start=(akt == 0), stop=(akt == ktc - 1),
                    skip_group_check=True)

            def emit_normalize(c, op):
                # denominator row 64 carries +n_pad from the padding keys.
                # The DVE part runs inline; the PE broadcast matmul is
                # deferred via the unit queue so it never stalls the PE
                # stream on the reciprocal.
                den = nrm_pool.tile([1, 512], F32, tag="den", name="den")
                nrm = nrm_pool.tile([1, 512], F32, tag="nrm", name="nrm")
                nc.vector.tensor_scalar_sub(den[:], op[64:65, :],
                                            npad_sb[0:1, 0:1])
                nc.vector.reciprocal(nrm[:], den[:])

                def bc_fn():
                    h, qc = c // QC, c % QC
                    hr, dst = (h * 64, otp_sb) if h < 2 else (0, ots_sb)
                    qsl = slice(qc * 512, (qc + 1) * 512)
                    g = gen_pool.tile([128, 512], F32, tag="gp", name="g")
                    nc.tensor.matmul(g[0:64, :], ones64[:], nrm[:],
                                     start=True, stop=True,
                                     skip_group_check=True)
                    bc = on_pool.tile([64, 512], F32, tag="on", name="bc")
                    nc.vector.tensor_copy(bc[:], g[0:64, :])
                    nc.vector.tensor_mul(dst[hr:hr + 64, qsl], op[0:64, :],
                                         bc[:])
                unit(("bc", c), bc_fn, front=True)

            seq = [(0, qc) for qc in range(QC)]
            seq += [(1, 0), (1, 1)]
            for qc in range(2, QC):
                seq += [(2, qc - 2), (1, qc)]
            seq += [(2, QC - 2), (2, QC - 1)]
            stream = [(h, qc, kt) for h, qc in seq for kt in range(ktc)]
            N = len(stream)
            state = {}
            for n in range(N + LA):
                if n < N:
                    h, qc, kt = stream[n]
                    ci = h * QC + qc
                    if kt == 0:
                        if h == 0 and qc + 1 < QC:
                            dma_xt(qc + 1)()
                    if kt == 8 and h == 0 and qc + 1 < QC:
                        unit(("q", 0, qc + 1), q_unit(False, qc + 1),
                             front=True)
                    if kt == 12 and h == 0:
                        unit(("q", 1, qc), q_unit(True, qc), front=True)
                    if kt == 0:
                        require(("q", 0, qc) if h < 2 else ("q", 1, qc))
                        state[ci] = {
                            "op": None, "sp": {}, "p": {}, "h": h, "qc": qc,
                        }
                    st = state[ci]
                    kkey = "kp" if h < 2 else "ks"
                    ksb = kp_sb if h < 2 else ks_sb
                    qsb = qp_sb if h < 2 else qs_sb
                    hr = h * 64 if h < 2 else 0
                    qsl = slice(qc * 512, (qc + 1) * 512)
                    gi = grp_of[kt]
                    kts = groups[gi]
                    if kt == kts[0]:
                        st["sp"][gi] = sp_pool.tile([128, WIDE * 512], F32,
                                                    tag="sp", name="sp")
                        st["p"][gi] = p_pool.tile([128, WIDE * 512], MMDT,
                                                  tag="p", name="pt")
                    require((kkey, kchunk_of(kt)))
                    sp = st["sp"][gi]
                    scol = (kt - kts[0]) * 512
                    nc.tensor.matmul(
                        sp[:, scol:scol + 512],
                        ksb[hr:hr + 64, kt * 128:(kt + 1) * 128],
                        qsb[hr:hr + 64, qsl],
                        start=True, stop=True, skip_group_check=True)
                    if kt == kts[-1]:
                        nc.scalar.activation(
                            st["p"][gi][:, 0:len(kts) * 512],
                            sp[:, 0:len(kts) * 512], AF.Exp,
                            bias=eb_sb[:, 0:1], scale=0.125)
                m = n - LA
                if m >= 0:
                    h2_, qc2_, akt = stream[m]
                    cj = h2_ * QC + qc2_
                    st = state[cj]
                    require(vkey(h2_, akt))
                    gj = grp_of[akt]
                    if st["op"] is None:
                        # allocated at first use: with bufs=1 the previous
                        # chunk's tail AVs must all be emitted before this
                        # alias of the same PSUM bank exists
                        st["op"] = op_pool.tile([65, 512], F32, tag="op",
                                                name="op")
                    emit_av(st["op"], h2_, st["p"][gj], gj, akt)
                    if akt == ktc - 1:
                        emit_normalize(cj, st["op"])
                        if h2_ == 2:
                            for mm in range(DT):
                                unit(("proj", qc2_, mm),
                                     proj_unit(qc2_, mm,
                                               use_sp=qc2_ >= QC - 1))
                        del state[cj]
                if (n % ktc) in (3, 6, 9, 12, 15):
                    pop_unit()
            # drain
            while queue:
                pop_unit()

        if reps == 1:
            emit_once()
        elif reps < 0:
            for _ in range(-reps):
                emit_once()
        else:
            with tc.For_i(0, reps, 1):
                emit_once()


# ---------------------------------------------------------------- host side

@functools.lru_cache(maxsize=None)
def _get_runner(reps: int, ktc: int):
    import jax
    from jax.sharding import Mesh, PartitionSpec
    from jax.experimental.shard_map import shard_map

    nc = build_program(reps, ktc)
    install_neuronx_cc_hook()
    partition_name = (nc.partition_id_tensor.name
                      if nc.partition_id_tensor else None)
    in_names, out_names, out_avals, out_shapes = [], [], [], []
    for alloc in nc.m.functions[0].allocations:
        if not isinstance(alloc, mybir.MemoryLocationSet):
            continue
        name = alloc.memorylocations[0].name
        if alloc.kind == "ExternalInput":
            if name != partition_name:
                in_names.append(name)
        elif alloc.kind == "ExternalOutput":
            out_names.append(name)
            shape = tuple(alloc.tensor_shape)
            dtype = mybir.dt.np(alloc.dtype)
            out_avals.append(jax.core.ShapedArray(shape, dtype))
            out_shapes.append((shape, dtype))
    n_params = len(in_names)
    n_outs = len(out_avals)
    all_in_names = list(in_names) + list(out_names)
    if partition_name is not None:
        all_in_names.append(partition_name)
    donate = tuple(range(n_params, n_params + n_outs))

    def _bodyf(*args):
        operands = list(args)
        if partition_name is not None:
            operands.append(partition_id_tensor())
        outs = _bass_exec_p.bind(
            *operands,
            out_avals=tuple(out_avals),
            in_names=tuple(all_in_names),
            out_names=tuple(out_names),
            lowering_input_output_aliases=(),
            sim_require_finite=True,
            sim_require_nnan=True,
            nc=nc,
        )
        return tuple(outs)

    devices = jax.devices()[:N_CORES]
    mesh = Mesh(np.asarray(devices), ("core",))
    in_specs = (PartitionSpec("core"),) * (n_params + n_outs)
    out_specs = (PartitionSpec("core"),) * len(out_names)
    sharded = jax.jit(
        shard_map(_bodyf, mesh=mesh, in_specs=in_specs, out_specs=out_specs,
                  check_rep=False),
        donate_argnums=donate, keep_unused=True,
    )

    def run(in_maps):
        import jax as _jax
        per_core = [[np.asarray(m[n]) for n in in_names] for m in in_maps]
        concat_in = [np.concatenate([per_core[c][i] for c in range(N_CORES)],
                                    axis=0) for i in range(n_params)]
        concat_zeros = [np.zeros((N_CORES * s[0], *s[1:]), dt)
                        for (s, dt) in out_shapes]
        out_arrs = sharded(*concat_in, *concat_zeros)
        _jax.block_until_ready(out_arrs)
        return [
            {name: np.asarray(out_arrs[i]).reshape(
                N_CORES, *out_shapes[i][0])[c]
             for i, name in enumerate(out_names)}
            for c in range(N_CORES)
        ]

    return run


def _plan_compaction(mask):
    mask = np.asarray(mask)
    keeps = [np.where(~mask[b])[0] for b in range(B)]
    ktc = max(1, -(-max(len(k) for k in keeps) // 128))
    return keeps, ktc


def make_in_maps(x, mask, w_qkv, w_proj, b_proj, keeps=None, ktc=None):
    import ml_dtypes
    mm_np = ml_dtypes.bfloat16
    x = np.asarray(x, np.float32)
    mask = np.asarray(mask)
    w_qkv = np.asarray(w_qkv, np.float32)
    w_proj = np.asarray(w_proj, np.float32)
    b_proj = np.asarray(b_proj, np.float32)
    if keeps is None:
        keeps, ktc = _plan_compaction(mask)
    tc_keys = ktc * 128

    xTs, xcTs, mbs = [], [], []
    for b in range(B):
        xTs.append(np.ascontiguousarray(x[b].T).astype(mm_np))
        xc = np.zeros((tc_keys, D), np.float32)
        xc[:len(keeps[b])] = x[b][keeps[b]]
        xcTs.append(np.ascontiguousarray(xc.T).astype(mm_np))
        # padding keys contribute exp(0/8 + EXP_SHIFT) each, as rounded to
        # fp8e4m3 by the activation output
        p_pad = float(np.float32(
            ml_dtypes.float8_e4m3(np.exp(np.float32(EXP_SHIFT)))))
        mbs.append(np.full((128, 1), (tc_keys - len(keeps[b])) * p_pad,
                           np.float32))

    bp_real = np.ascontiguousarray(b_proj.reshape(DT, 128).T)
    bp_zero = np.zeros_like(bp_real)

    in_maps = []
    for c in range(N_CORES):
        b, hg = divmod(c, 4)
        hs = [hg * HPC + i for i in range(HPC)]
        qrows = np.r_[hs[0] * 64:(hs[0] + 1) * 64, hs[1] * 64:(hs[1] + 1) * 64]
        srow = slice(hs[2] * 64, (hs[2] + 1) * 64)
        feat = np.r_[tuple(np.r_[h * 64:(h + 1) * 64] for h in hs)]
        in_maps.append({
            "xT": xTs[b],
            "xcT": xcTs[b],
            "wqp": np.ascontiguousarray(w_qkv[qrows, :].T).astype(mm_np),
            "wqs": np.ascontiguousarray(w_qkv[srow, :].T).astype(mm_np),
            "wkp": np.ascontiguousarray(w_qkv[D + qrows, :].T).astype(mm_np),
            "wks": np.ascontiguousarray(
                w_qkv[D + hs[2] * 64:D + (hs[2] + 1) * 64, :].T).astype(mm_np),
            "wv": np.ascontiguousarray(
                w_qkv[2 * D + feat, :].T).astype(mm_np),
            "wpp": np.ascontiguousarray(w_proj[:, feat[:128]].T).astype(mm_np),
            "wps": np.ascontiguousarray(w_proj[:, feat[128:]].T).astype(mm_np),
            "bp": bp_real if hg == 0 else bp_zero,
            "npad": mbs[b],
        })
    return in_maps


def assemble_output(results):
    out = np.empty((B, T, D), np.float32)
    for b in range(B):
        acc = results[4 * b]["outT"].astype(np.float32)
        for hg in range(1, 4):
            acc = acc + results[4 * b + hg]["outT"]
        out[b] = acc.T
    return out


def kernel(x, mask, w_qkv, w_proj, b_proj):
    keeps, ktc = _plan_compaction(mask)
    run = _get_runner(1, ktc)
    in_maps = make_in_maps(x, mask, w_qkv, w_proj, b_proj, keeps, ktc)
    results = run(in_maps)
    return assemble_output(results)
